# revision 17
# baseline (speedup 1.0000x reference)
"""AttentionRPE kernel for 8 Trainium2 NeuronCores.

Math (per (b,s) row, T=128 targets, D=256, H=8 heads, DH=32, DR=32):
  q   = src @ Wsrc.T + bsrc                       [D]
  K'  = tgt @ Wk.T + rpe @ Rwk.T (+const bias)    [T, D]
  V'  = tgt @ Wv.T + rpe @ Rwv.T (+const bias)    [T, D]
  att = softmax_h(q_h . K'_h / sqrt(DH))          [H, T]   (masked)
  out = (att @ V')_heads @ Wout.T + bout          [D]

Device formulation:
  * K-path q-fold: logits[h,t] = sum_d qw[h,d]*tgtx[t,d], with
    qw = (q/sqrt(DH)) @ Wkx  folded per row (tiny), tgtx = [tgt | rpe].
    qw for all (s,h) columns is computed with dense N=512 matmuls using
    a zero-padded q tensor (qexp[e,(s,h)] = q[s,e] iff e in head h).
  * The big tensor is shipped in BOTH layouts, fp16, prepared on host:
    natural [t, d] for the V-path and transposed [d, (s,t)] for the
    K-path.  Same total bytes as one fp32 copy; zero on-chip transposes
    of tgtx.
  * V-path: per 4-row group, 4 accumulating matmuls with zero-padded
    attention stationaries (avz) compute GT[(s,h), d] for 4 rows at a
    time into one [128,288] PSUM tile (col-tiled per group); GT is then
    transposed (3 eye-matmuls per 16-row block) into the G[d,(s,h)]
    layout that feeds the output projection, where
    Wfx[h] = (Wout[:,hslice] @ Wvx[hslice,:]).T is precomputed on host.
  * All on-chip transposes are regular matmuls against a fp16 identity
    (never transpose-mode): transpose-mode activity does not count as
    PE-busy for the HAM clock gate and keeps the PE at 1.2 GHz.  A
    dummy-matmul warmup burst flips the clock gate before real work.
  * Padding mask + off-diagonal garbage masking folded into one extra
    accumulating matmul into the logits PSUM (rank-16 selector A16 @ Bm);
    the Bm rows ride along in the transposed-layout DMA.  The all-rows-
    invalid row mask is folded into the softmax denominator; the output
    bias is host-masked.
  * Main loop is software-pipelined: block k's logits are emitted before
    block k-1's attention-transpose/V-path so the PE never stalls on the
    softmax chain.

Sharding: 1024 (b,s) rows split contiguously over 8 cores (128 each).
"""

import numpy as np

import concourse.bass as bass
import concourse.bacc as bacc
import concourse.mybir as mybir
from concourse.tile import TileContext
from concourse.masks import make_identity
from concourse.bass_utils import run_bass_kernel_spmd

B, S, T, D = 2, 512, 128, 256
H, DH, DR = 8, 32, 32
DX = D + DR          # 288 = tgt|rpe feature dim
DXM = DX + 16        # 304 = transposed layout rows (288 features + 16 bm rows)
DOUT = D
NCORES = 8
BS = B * S           # 1024 total rows
SC = BS // NCORES    # 128 rows per core
MASKV = -60000.0     # fits fp16; exp() still underflows to exactly 0

F32 = mybir.dt.float32
F16 = mybir.dt.float16

AX = mybir.AxisListType
ALU = mybir.AluOpType
ACTF = mybir.ActivationFunctionType


def build(sc=SC):
    """Build the per-core Bass program. sc = rows per core (multiple of 16)."""
    assert sc % 16 == 0
    nblk = sc // 16
    ngr = sc // 4
    nc = bacc.Bacc()

    src_d = nc.dram_tensor("src", [sc, D], F16, kind="ExternalInput")
    wkk_d = nc.dram_tensor("wkk", [D, 544], F16, kind="ExternalInput")
    fbl_d = nc.dram_tensor("fbl", [128, 10], F32, kind="ExternalInput")
    eye_d = nc.dram_tensor("eye", [128, 128], F16, kind="ExternalInput")
    txn_d = nc.dram_tensor("txn", [ngr, T, 4 * DX], F16, kind="ExternalInput")
    txt_d = nc.dram_tensor("txt", [ngr, DXM, 512], F16, kind="ExternalInput")
    a16x_d = nc.dram_tensor("a16x", [16, sc, H], F16, kind="ExternalInput")
    wfx_d = nc.dram_tensor("wfx", [H, DX, DOUT], F16, kind="ExternalInput")
    obias_d = nc.dram_tensor("obias", [sc, DOUT], F32, kind="ExternalInput")
    out_d = nc.dram_tensor("out", [sc, DOUT], F32, kind="ExternalOutput")

    with TileContext(nc) as tc:
        with (
            tc.tile_pool(name="const", bufs=1) as cp,
            tc.tile_pool(name="txp", bufs=14) as txp,
            tc.tile_pool(name="txtp", bufs=12) as txtp,
            tc.tile_pool(name="attnp", bufs=2) as attnp,
            tc.tile_pool(name="smallp", bufs=2) as smallp,
            tc.tile_pool(name="ps_l", bufs=2, space="PSUM") as ps_l,
            tc.tile_pool(name="ps_atr", bufs=2, space="PSUM") as ps_atr,
            tc.tile_pool(name="ps_g", bufs=1, space="PSUM") as ps_g,
            tc.tile_pool(name="ps_misc", bufs=1, space="PSUM") as ps_misc,
        ):
            # ---------------- early inputs ----------------
            eye16 = cp.tile([128, 128], F16, name="eye16")
            nc.sync.dma_start(out=eye16, in_=eye_d[:, :])
            src_sb = cp.tile([sc, D], F16, name="src_sb")
            nc.sync.dma_start(out=src_sb, in_=src_d[:, :])
            wkk = []
            for c in range(2):
                wt = cp.tile([128, 544], F16, name=f"wkk{c}")
                nc.sync.dma_start(out=wt, in_=wkk_d[c * 128:(c + 1) * 128, :])
                wkk.append(wt)
            wsrcT = [wkk[c][:, 0:D] for c in range(2)]
            wk = [wkk[c][:, D:2 * D] for c in range(2)]
            rwk = [wkk[c][:, 2 * D:2 * D + DR] for c in range(2)]
            fbl = cp.tile([128, 10], F32, name="fbl")
            nc.sync.dma_start(out=fbl, in_=fbl_d[:, :])
            bsrc = [fbl[:, c:c + 1] for c in range(2)]
            rmaskx = fbl[:, 2:10]

            # HAM warmup: dependency-free matmuls flip the PE clock gate to
            # 8/8 before real work arrives (~3.4us of sustained activity).
            warm_ps = ps_misc.tile([128, 128], F32, tag="mA", name="warm_ps",
                                   bufs=2)
            for i in range(64):
                nc.tensor.matmul(warm_ps, eye16, eye16, start=True, stop=True,
                                 skip_group_check=True)

            gall = cp.tile([128, sc, 24], F16, name="gall")
            avz = []
            for i in range(2):
                az = cp.tile([128, 16, 32], F16, name=f"avz{i}")
                nc.gpsimd.memset(az, 0.0)
                avz.append(az)
            qexp = []
            for ti in range(2):
                qx = cp.tile([128, sc, H], F16, name=f"qexp{ti}")
                nc.gpsimd.memset(qx, 0.0)
                qexp.append(qx)
            qexpf = [t.rearrange("p s h -> p (s h)") for t in qexp]

            # ---------------- q path (once per core) ----------------
            srcT = []
            for c in range(2):
                st_ps = ps_misc.tile([128, sc], F32, tag="mB", name="st_ps")
                nc.tensor.matmul(st_ps, src_sb[:, c * 128:(c + 1) * 128],
                                 eye16[0:sc, 0:sc], start=True, stop=True)
                st = cp.tile([128, sc], F16, name=f"srcT{c}")
                nc.vector.tensor_copy(st, st_ps)
                srcT.append(st)
            # q (+bias) written per head into the zero-padded qexp
            for ec in range(2):
                q_ps = ps_misc.tile([128, sc], F32, tag="mB", name="q_ps")
                for dc in range(2):
                    nc.tensor.matmul(
                        q_ps,
                        wsrcT[dc][:, ec * 128:(ec + 1) * 128],
                        srcT[dc],
                        start=(dc == 0), stop=(dc == 1))
                for hh in range(4):
                    ro = hh * 32
                    h = ec * 4 + hh
                    nc.vector.tensor_scalar_add(
                        qexp[ec][ro:ro + 32, :, h],
                        q_ps[ro:ro + 32, :], bsrc[ec][ro:ro + 32, :])
            # qw[d,(s,h)] / qrw[r,(s,h)] via dense matmuls against qexp
            qwT = []
            for dc in range(2):
                qwT.append(cp.tile([128, sc, H], F16, name=f"qwT{dc}"))
            qwTf = [t.rearrange("p s h -> p (s h)") for t in qwT]
            qrwT = cp.tile([48, sc, H], F16, name="qrwT")
            qrwTf = qrwT.rearrange("p s h -> p (s h)")
            nc.sync.dma_start(out=qrwT[32:48, :, :], in_=a16x_d[:, :, :])
            for dc in range(2):
                for cb in range(2):
                    qw_ps = ps_misc.tile([128, 512], F32, tag="mA",
                                         name="qw_ps", bufs=2)
                    for ti in range(2):
                        nc.tensor.matmul(
                            qw_ps,
                            wk[ti][:, dc * 128:(dc + 1) * 128],
                            qexpf[ti][:, cb * 512:(cb + 1) * 512],
                            start=(ti == 0), stop=(ti == 1))
                    nc.vector.tensor_copy(
                        qwTf[dc][:, cb * 512:(cb + 1) * 512], qw_ps)
            for cb in range(2):
                qr_ps = ps_misc.tile([32, 512], F32, tag="mB", name="qr_ps")
                for ti in range(2):
                    nc.tensor.matmul(
                        qr_ps,
                        rwk[ti],
                        qexpf[ti][:, cb * 512:(cb + 1) * 512],
                        start=(ti == 0), stop=(ti == 1))
                nc.scalar.activation(
                    qrwTf[0:32, cb * 512:(cb + 1) * 512], qr_ps, ACTF.Copy)

            # late-needed weights (issued mid-loop; DMA has slack there)
            wfx_main = {}
            wfx_r = {}
            obias = cp.tile([sc, DOUT], F32, name="obias")

            def issue_wfx(h):
                for c in range(2):
                    wt = cp.tile([128, DOUT], F16, name=f"wfx{h}_{c}")
                    nc.scalar.dma_start(
                        out=wt, in_=wfx_d[h, c * 128:(c + 1) * 128, :])
                    wfx_main[(h, c)] = wt
                wt = cp.tile([32, DOUT], F16, name=f"wfxr{h}")
                nc.scalar.dma_start(out=wt, in_=wfx_d[h, D:DX, :])
                wfx_r[h] = wt

            # ---------------- pipelined main loop ----------------
            tx_blk = {}     # blk -> list of 4 natural-layout tiles
            attn_blk = {}   # blk -> normalized attention tile

            def stageA(blk):
                """DMAs + logits matmuls for block blk."""
                l_ps = ps_l.tile([128, 512], F32, name="l_ps")
                tx_tiles = []
                t2p = None
                for g in range(4):
                    gi = blk * 4 + g
                    t01 = txtp.tile([128, 2, 512], F16, tag="t01", name="t01")
                    nc.sync.dma_start(
                        out=t01,
                        in_=txt_d[gi, 0:256, :].rearrange(
                            "(c p) x -> p c x", c=2))
                    if g % 2 == 0:
                        t2p = txtp.tile([48, 2, 512], F16, tag="t2", name="t2p")
                        nc.sync.dma_start(
                            out=t2p,
                            in_=txt_d[gi:gi + 2, 256:304, :].rearrange(
                                "g p x -> p g x"))
                    tx4 = txp.tile([T, 4, DX], F16, tag="tx", name="tx4")
                    nc.scalar.dma_start(
                        out=tx4.rearrange("t f d -> t (f d)"),
                        in_=txn_d[gi, :, :])
                    tx_tiles.append(tx4)
                    osl = slice(g * 32, (g + 1) * 32)
                    nc.tensor.matmul(
                        l_ps[osl, :],
                        qwTf[0][:, gi * 32:gi * 32 + 32],
                        t01[:, 0, :], start=True, stop=False,
                        tile_position=(0, g * 32))
                    nc.tensor.matmul(
                        l_ps[osl, :],
                        qwTf[1][:, gi * 32:gi * 32 + 32],
                        t01[:, 1, :], start=False, stop=False,
                        tile_position=(0, g * 32))
                    nc.tensor.matmul(
                        l_ps[osl, :],
                        qrwTf[:, gi * 32:gi * 32 + 32],
                        t2p[:, g % 2, :], start=False, stop=True,
                        tile_position=(0, g * 32))
                tx_blk[blk] = tx_tiles
                return l_ps

            def stageB(blk, l_ps):
                """Masked softmax for block blk (row mask folded into 1/den)."""
                nmx = smallp.tile([128, 1], F32, tag="nmx", name="nmx")
                nc.vector.tensor_reduce(nmx, l_ps, axis=AX.X, op=ALU.max,
                                        negate=True)
                den = smallp.tile([128, 1], F32, tag="den", name="den")
                attn_e = attnp.tile([128, 512], F16, tag="ae", name="attn_e")
                nc.scalar.activation(attn_e, l_ps, ACTF.Exp, bias=nmx,
                                     scale=1.0, accum_out=den)
                rden = smallp.tile([128, 1], F32, tag="rden", name="rden")
                nc.vector.reciprocal(rden, den)
                rdm = smallp.tile([128, 1], F32, tag="rdm", name="rdm")
                nc.vector.tensor_tensor(rdm, rden, rmaskx[:, blk:blk + 1],
                                        op=ALU.mult)
                attn_n = attnp.tile([128, 512], F16, tag="an", name="attn_n")
                nc.vector.tensor_scalar_mul(attn_n, attn_e, rdm)
                attn_blk[blk] = attn_n

            def stageC(blk):
                """Attention transpose + V-path for block blk."""
                attn_n = attn_blk.pop(blk)
                tx_tiles = tx_blk.pop(blk)
                at4_ps = ps_atr.tile([128, 4, 128], F32, tag="atr",
                                     name="at4_ps")
                for g in range(4):
                    nc.tensor.matmul(
                        at4_ps[:, g, :],
                        attn_n[:, g * 128:(g + 1) * 128],
                        eye16,
                        start=True, stop=True)
                az = avz[blk % 2]
                at4v = at4_ps.rearrange("p g (s h) -> p g s h", h=8)
                for gc in range(4):
                    nc.vector.tensor_copy(
                        az[:, gc::4, 8 * gc:8 * gc + 8],
                        at4v[:, gc, gc::4, :])
                gt2_ps = ps_g.tile([128, DX], F32, tag="gt", name="gt2_ps")
                for g in range(4):
                    for kk in range(4):
                        nc.tensor.matmul(
                            gt2_ps[32 * g:32 * (g + 1), :],
                            az[:, 4 * g + kk, :],
                            tx_tiles[g][:, kk, :],
                            start=(kk == 0), stop=(kk == 3),
                            tile_position=(0, 32 * g))
                gt2sb = smallp.tile([128, DX], F16, tag="g2", name="gt2sb")
                nc.scalar.activation(gt2sb, gt2_ps, ACTF.Copy)
                bsl = slice(blk * 16, (blk + 1) * 16)
                for c in range(2):
                    tr = ps_atr.tile([128, 128], F32, tag="atr", name="tr")
                    nc.tensor.matmul(tr, gt2sb[:, c * 128:(c + 1) * 128],
                                     eye16, start=True, stop=True)
                    nc.vector.tensor_copy(
                        gall[:, bsl, c * 8:(c + 1) * 8],
                        tr.rearrange("p (s h) -> p s h", h=8))
                tr2 = ps_atr.tile([32, 128], F32, tag="atr", name="tr2")
                nc.tensor.matmul(tr2, gt2sb[:, 256:288], eye16,
                                 start=True, stop=True)
                nc.vector.tensor_copy(
                    gall[0:32, bsl, 16:24],
                    tr2.rearrange("p (s h) -> p s h", h=8))

            def heartbeat(n):
                for _ in range(n):
                    nc.tensor.matmul(warm_ps, eye16, eye16, start=True,
                                     stop=True, skip_group_check=True)

            for blk in range(nblk):
                if blk > 0:
                    heartbeat(12)
                l_ps = stageA(blk)
                if 1 <= blk <= 4:
                    for h in range(2 * (blk - 1), 2 * blk):
                        issue_wfx(h)
                if blk == 3:
                    nc.scalar.dma_start(out=obias, in_=obias_d[:, :])
                stageB(blk, l_ps)
                if blk >= 1:
                    stageC(blk - 1)
            heartbeat(20)
            stageC(nblk - 1)

            # ---------------- output projection ----------------
            out_ps = ps_misc.tile([sc, DOUT], F32, tag="mA", name="out_ps",
                                  bufs=2)
            for h in range(H):
                for c in range(2):
                    nc.tensor.matmul(
                        out_ps,
                        gall[:, :, c * 8 + h],
                        wfx_main[(h, c)],
                        start=(h == 0 and c == 0), stop=False)
                nc.tensor.matmul(
                    out_ps,
                    gall[0:32, :, 16 + h],
                    wfx_r[h],
                    start=False, stop=(h == H - 1))
            out_sb = cp.tile([sc, DOUT], F32, name="out_sb")
            nc.vector.tensor_tensor(out_sb, out_ps, obias, op=ALU.add)
            nc.sync.dma_start(out=out_d[:, :], in_=out_sb)

    nc.finalize()
    return nc


def host_prep(src, tgt, rpe, tgt_padding_mask, in_proj_weight, in_proj_bias,
              out_proj_weight, out_proj_bias, rpe_weight, rpe_bias):
    """Host-side slicing/weight prep. Returns per-core input maps."""
    f = np.float32
    f16 = np.float16
    scale = f(1.0 / np.sqrt(DH))
    src_f = np.ascontiguousarray(np.asarray(src, f).reshape(BS, D)).astype(f16)
    tgtx = np.concatenate(
        [np.asarray(tgt, f).reshape(BS, T, D),
         np.asarray(rpe, f).reshape(BS, T, DR)], axis=-1).astype(f16)
    mask = np.asarray(tgt_padding_mask, bool).reshape(BS, T)
    no_valid = mask.all(-1)
    maskadd = np.where(mask & ~no_valid[:, None], f16(MASKV), f16(0.0))
    rowmask = (~no_valid).astype(f)

    # bm[blk, j, :] : row j's mask at its own group slot, MASKV elsewhere
    nblk_total = BS // 16
    bm = np.full((nblk_total, 16, 4, T), f16(MASKV), f16)
    ma_b = maskadd.reshape(nblk_total, 16, T)
    for j in range(16):
        bm[:, j, j % 4, :] = ma_b[:, j, :]
    bm = bm.reshape(nblk_total, 16, 512)

    # natural layout, group-packed: [ngr, T, 4*DX]
    ngr_total = BS // 4
    txn = np.ascontiguousarray(
        tgtx.reshape(ngr_total, 4, T, DX).transpose(0, 2, 1, 3)
    ).reshape(ngr_total, T, 4 * DX)
    # transposed layout + bm rows: [ngr, DXM, 512]
    txt = np.empty((ngr_total, DXM, 512), f16)
    txt[:, :DX, :] = tgtx.reshape(ngr_total, 4, T, DX).transpose(
        0, 3, 1, 2).reshape(ngr_total, DX, 512)
    txt[:, DX:, :] = bm[np.arange(ngr_total) // 4]

    sidx = np.arange(SC) % 16
    a16x = (np.arange(16)[:, None, None] == sidx[None, :, None]).astype(f16)
    a16x = np.ascontiguousarray(np.broadcast_to(a16x, (16, SC, H)))

    ipw = np.asarray(in_proj_weight, f)
    ipb = np.asarray(in_proj_bias, f)
    opw = np.asarray(out_proj_weight, f)
    opb = np.asarray(out_proj_bias, f)
    rw = np.asarray(rpe_weight, f)
    rb = np.asarray(rpe_bias, f)

    wsrcT = (ipw[:D].T * scale).astype(f16)                      # [d, e]
    bsrc = (ipb[:D] * scale).astype(f)                           # [D]
    wk = ipw[D:2 * D].astype(f16)                                # [e, d]
    rwk = rw[:D].astype(f16)                                     # [e, r]
    wkk = np.ascontiguousarray(
        np.concatenate([wsrcT, wk, rwk], axis=1))                # [256, 544]
    wvx = np.concatenate([ipw[2 * D:3 * D], rw[D:2 * D]], axis=1)  # [e, 288]
    wfx = np.empty((H, DX, DOUT), f)
    for h in range(H):
        hs = slice(h * 32, (h + 1) * 32)
        wfx[h] = (opw[:, hs] @ wvx[hs, :]).T
    wfx = wfx.astype(f16)
    obias_full = (opb + opw @ (ipb[2 * D:3 * D] + rb[D:2 * D])).astype(f)
    obias_all = np.ascontiguousarray(
        rowmask[:, None] * obias_full[None, :]).astype(f)

    # rmaskx[j*8+h, blk] = rowmask for row (blk*16+j), per core
    nblk = SC // 16
    ngr = SC // 4
    in_maps = []
    for c in range(NCORES):
        sl = slice(c * SC, (c + 1) * SC)
        rm = rowmask[sl].reshape(nblk, 16)
        rmx = np.repeat(rm.T, H, axis=0).astype(f)    # [128, nblk]
        fbl = np.ascontiguousarray(np.concatenate(
            [bsrc.reshape(2, 128).T, rmx], axis=1))   # [128, 10]
        in_maps.append({
            "src": src_f[sl],
            "txn": txn[c * ngr:(c + 1) * ngr],
            "txt": txt[c * ngr:(c + 1) * ngr],
            "a16x": a16x,
            "wkk": wkk,
            "wfx": wfx,
            "fbl": fbl,
            "eye": np.eye(128, dtype=f16),
            "obias": obias_all[sl],
        })
    return in_maps


_NC_CACHE = {}


def get_nc(sc=SC):
    if sc not in _NC_CACHE:
        _NC_CACHE[sc] = build(sc)
    return _NC_CACHE[sc]


def run(in_maps, trace=False):
    nc = get_nc(SC)
    return run_bass_kernel_spmd(nc, in_maps, list(range(NCORES)), trace=trace)


def kernel(**inputs):
    in_maps = host_prep(**inputs)
    res = run(in_maps).results
    out = np.concatenate([res[c]["out"] for c in range(NCORES)], axis=0)
    return np.ascontiguousarray(out.reshape(B, S, D))


# revision 18
# speedup vs baseline: 1.0703x; 1.0703x over previous
"""AttentionRPE kernel for 8 Trainium2 NeuronCores.

Math (per (b,s) row, T=128 targets, D=256, H=8 heads, DH=32, DR=32):
  q   = src @ Wsrc.T + bsrc                       [D]
  K'  = tgt @ Wk.T + rpe @ Rwk.T (+const bias)    [T, D]
  V'  = tgt @ Wv.T + rpe @ Rwv.T (+const bias)    [T, D]
  att = softmax_h(q_h . K'_h / sqrt(DH))          [H, T]   (masked)
  out = (att @ V')_heads @ Wout.T + bout          [D]

Device formulation:
  * K-path q-fold: logits[h,t] = sum_d qw[h,d]*tgtx[t,d], with
    qw = (q/sqrt(DH)) @ Wkx  folded per row (tiny), tgtx = [tgt | rpe].
    qw for all (s,h) columns is computed with dense N=512 matmuls using
    a zero-padded q tensor (qexp[e,(s,h)] = q[s,e] iff e in head h).
  * The big tensor is shipped in BOTH layouts, fp16, prepared on host:
    natural [t, d] for the V-path and transposed [d, (s,t)] for the
    K-path.  Same total bytes as one fp32 copy; zero on-chip transposes
    of tgtx.
  * V-path: per 4-row group, 4 accumulating matmuls with zero-padded
    attention stationaries (avz) compute GT[(s,h), d] for 4 rows at a
    time into one [128,288] PSUM tile (col-tiled per group); GT is then
    transposed (3 eye-matmuls per 16-row block) into the G[d,(s,h)]
    layout that feeds the output projection, where
    Wfx[h] = (Wout[:,hslice] @ Wvx[hslice,:]).T is precomputed on host.
  * All on-chip transposes are regular matmuls against a fp16 identity
    (never transpose-mode): transpose-mode activity does not count as
    PE-busy for the HAM clock gate and keeps the PE at 1.2 GHz.  A
    dummy-matmul warmup burst flips the clock gate before real work.
  * Padding mask + off-diagonal garbage masking folded into one extra
    accumulating matmul into the logits PSUM (rank-16 selector A16 @ Bm);
    the Bm rows ride along in the transposed-layout DMA.  The all-rows-
    invalid row mask is folded into the softmax denominator; the output
    bias is host-masked.
  * Main loop is software-pipelined: block k's logits are emitted before
    block k-1's attention-transpose/V-path so the PE never stalls on the
    softmax chain.

Sharding: 1024 (b,s) rows split contiguously over 8 cores (128 each).
"""

import numpy as np

import concourse.bass as bass
import concourse.bacc as bacc
import concourse.mybir as mybir
from concourse.tile import TileContext
from concourse.masks import make_identity
from concourse.bass_utils import run_bass_kernel_spmd

B, S, T, D = 2, 512, 128, 256
H, DH, DR = 8, 32, 32
DX = D + DR          # 288 = tgt|rpe feature dim
DXM = DX + 16        # 304 = transposed layout rows (288 features + 16 bm rows)
DOUT = D
NCORES = 8
BS = B * S           # 1024 total rows
SC = BS // NCORES    # 128 rows per core
MASKV = -60000.0     # fits fp16; exp() still underflows to exactly 0

F32 = mybir.dt.float32
F16 = mybir.dt.float16

AX = mybir.AxisListType
ALU = mybir.AluOpType
ACTF = mybir.ActivationFunctionType


def build(sc=SC):
    """Build the per-core Bass program. sc = rows per core (multiple of 16)."""
    assert sc % 16 == 0
    nblk = sc // 16
    ngr = sc // 4
    nc = bacc.Bacc()

    src_d = nc.dram_tensor("src", [sc, D], F16, kind="ExternalInput")
    wkk_d = nc.dram_tensor("wkk", [D, 544], F16, kind="ExternalInput")
    fbl_d = nc.dram_tensor("fbl", [128, 10], F32, kind="ExternalInput")
    eye_d = nc.dram_tensor("eye", [128, 128], F16, kind="ExternalInput")
    txn_d = nc.dram_tensor("txn", [ngr, T, 4 * DX], F16, kind="ExternalInput")
    txt_d = nc.dram_tensor("txt", [ngr, DXM, 512], F16, kind="ExternalInput")
    a16x_d = nc.dram_tensor("a16x", [16, sc, H], F16, kind="ExternalInput")
    wfx_d = nc.dram_tensor("wfx", [H, DX, DOUT], F16, kind="ExternalInput")
    obias_d = nc.dram_tensor("obias", [sc, DOUT], F32, kind="ExternalInput")
    out_d = nc.dram_tensor("out", [sc, DOUT], F32, kind="ExternalOutput")

    with TileContext(nc) as tc:
        with (
            tc.tile_pool(name="const", bufs=1) as cp,
            tc.tile_pool(name="txp", bufs=12) as txp,
            tc.tile_pool(name="txtp", bufs=10) as txtp,
            tc.tile_pool(name="attnp", bufs=2) as attnp,
            tc.tile_pool(name="smallp", bufs=2) as smallp,
            tc.tile_pool(name="ps_l", bufs=2, space="PSUM") as ps_l,
            tc.tile_pool(name="ps_atr", bufs=2, space="PSUM") as ps_atr,
            tc.tile_pool(name="ps_g", bufs=1, space="PSUM") as ps_g,
            tc.tile_pool(name="ps_misc", bufs=1, space="PSUM") as ps_misc,
        ):
            # ---------------- early inputs ----------------
            eye16 = cp.tile([128, 128], F16, name="eye16")
            nc.sync.dma_start(out=eye16, in_=eye_d[:, :])
            src_sb = cp.tile([sc, D], F16, name="src_sb")
            nc.sync.dma_start(out=src_sb, in_=src_d[:, :])
            wkk = []
            for c in range(2):
                wt = cp.tile([128, 544], F16, name=f"wkk{c}")
                nc.sync.dma_start(out=wt, in_=wkk_d[c * 128:(c + 1) * 128, :])
                wkk.append(wt)
            wsrcT = [wkk[c][:, 0:D] for c in range(2)]
            wk = [wkk[c][:, D:2 * D] for c in range(2)]
            rwk = [wkk[c][:, 2 * D:2 * D + DR] for c in range(2)]
            fbl = cp.tile([128, 10], F32, name="fbl")
            nc.sync.dma_start(out=fbl, in_=fbl_d[:, :])
            bsrc = [fbl[:, c:c + 1] for c in range(2)]
            rmaskx = fbl[:, 2:10]

            # HAM warmup: dependency-free matmuls flip the PE clock gate to
            # 8/8 before real work arrives (~3.4us of sustained activity).
            warm_ps = ps_misc.tile([128, 128], F32, tag="mA", name="warm_ps",
                                   bufs=2)
            for i in range(64):
                nc.tensor.matmul(warm_ps, eye16, eye16, start=True, stop=True,
                                 skip_group_check=True)

            gall = cp.tile([128, sc, 24], F16, name="gall")
            avz = []
            for i in range(2):
                az = cp.tile([128, 16, 32], F16, name=f"avz{i}")
                nc.gpsimd.memset(az, 0.0)
                avz.append(az)
            qexp = []
            for ti in range(2):
                qx = cp.tile([128, sc, H], F16, name=f"qexp{ti}")
                nc.gpsimd.memset(qx, 0.0)
                qexp.append(qx)
            qexpf = [t.rearrange("p s h -> p (s h)") for t in qexp]

            # ---------------- q path (once per core) ----------------
            srcT = []
            for c in range(2):
                st_ps = ps_misc.tile([128, sc], F32, tag="mB", name="st_ps")
                nc.tensor.matmul(st_ps, src_sb[:, c * 128:(c + 1) * 128],
                                 eye16[0:sc, 0:sc], start=True, stop=True)
                st = cp.tile([128, sc], F16, name=f"srcT{c}")
                nc.vector.tensor_copy(st, st_ps)
                srcT.append(st)
            # q (+bias) written per head into the zero-padded qexp
            for ec in range(2):
                q_ps = ps_misc.tile([128, sc], F32, tag="mB", name="q_ps")
                for dc in range(2):
                    nc.tensor.matmul(
                        q_ps,
                        wsrcT[dc][:, ec * 128:(ec + 1) * 128],
                        srcT[dc],
                        start=(dc == 0), stop=(dc == 1))
                for hh in range(4):
                    ro = hh * 32
                    h = ec * 4 + hh
                    nc.vector.tensor_scalar_add(
                        qexp[ec][ro:ro + 32, :, h],
                        q_ps[ro:ro + 32, :], bsrc[ec][ro:ro + 32, :])
            # qw[d,(s,h)] / qrw[r,(s,h)] via dense matmuls against qexp
            qwT = []
            for dc in range(2):
                qwT.append(cp.tile([128, sc, H], F16, name=f"qwT{dc}"))
            qwTf = [t.rearrange("p s h -> p (s h)") for t in qwT]
            qrwT = cp.tile([48, sc, H], F16, name="qrwT")
            qrwTf = qrwT.rearrange("p s h -> p (s h)")
            nc.sync.dma_start(out=qrwT[32:48, :, :], in_=a16x_d[:, :, :])
            for dc in range(2):
                for cb in range(2):
                    qw_ps = ps_misc.tile([128, 512], F32, tag="mA",
                                         name="qw_ps", bufs=2)
                    for ti in range(2):
                        nc.tensor.matmul(
                            qw_ps,
                            wk[ti][:, dc * 128:(dc + 1) * 128],
                            qexpf[ti][:, cb * 512:(cb + 1) * 512],
                            start=(ti == 0), stop=(ti == 1))
                    nc.vector.tensor_copy(
                        qwTf[dc][:, cb * 512:(cb + 1) * 512], qw_ps)
            for cb in range(2):
                qr_ps = ps_misc.tile([32, 512], F32, tag="mB", name="qr_ps")
                for ti in range(2):
                    nc.tensor.matmul(
                        qr_ps,
                        rwk[ti],
                        qexpf[ti][:, cb * 512:(cb + 1) * 512],
                        start=(ti == 0), stop=(ti == 1))
                nc.scalar.activation(
                    qrwTf[0:32, cb * 512:(cb + 1) * 512], qr_ps, ACTF.Copy)

            # late-needed weights (issued mid-loop; DMA has slack there)
            wfx_main = {}
            wfx_r = {}
            obias = cp.tile([sc, DOUT], F32, name="obias")

            def issue_wfx(h):
                for c in range(2):
                    wt = cp.tile([128, DOUT], F16, name=f"wfx{h}_{c}")
                    nc.scalar.dma_start(
                        out=wt, in_=wfx_d[h, c * 128:(c + 1) * 128, :])
                    wfx_main[(h, c)] = wt
                wt = cp.tile([32, DOUT], F16, name=f"wfxr{h}")
                nc.scalar.dma_start(out=wt, in_=wfx_d[h, D:DX, :])
                wfx_r[h] = wt

            # ---------------- pipelined main loop ----------------
            tx_blk = {}     # blk -> list of 4 natural-layout tiles
            attn_blk = {}   # blk -> normalized attention tile

            def stageA(blk):
                """DMAs + logits matmuls for block blk."""
                l_ps = ps_l.tile([128, 512], F32, name="l_ps")
                tx_tiles = []
                t2p = None
                for g in range(4):
                    gi = blk * 4 + g
                    t01 = txtp.tile([128, 2, 512], F16, tag="t01", name="t01")
                    nc.sync.dma_start(
                        out=t01,
                        in_=txt_d[gi, 0:256, :].rearrange(
                            "(c p) x -> p c x", c=2))
                    if g % 2 == 0:
                        t2p = txtp.tile([48, 2, 512], F16, tag="t2", name="t2p")
                        nc.sync.dma_start(
                            out=t2p,
                            in_=txt_d[gi:gi + 2, 256:304, :].rearrange(
                                "g p x -> p g x"))
                    tx4 = txp.tile([T, 4, DX], F16, tag="tx", name="tx4")
                    nc.scalar.dma_start(
                        out=tx4.rearrange("t f d -> t (f d)"),
                        in_=txn_d[gi, :, :])
                    tx_tiles.append(tx4)
                    osl = slice(g * 32, (g + 1) * 32)
                    nc.tensor.matmul(
                        l_ps[osl, :],
                        qwTf[0][:, gi * 32:gi * 32 + 32],
                        t01[:, 0, :], start=True, stop=False,
                        tile_position=(0, g * 32))
                    nc.tensor.matmul(
                        l_ps[osl, :],
                        qwTf[1][:, gi * 32:gi * 32 + 32],
                        t01[:, 1, :], start=False, stop=False,
                        tile_position=(0, g * 32))
                    nc.tensor.matmul(
                        l_ps[osl, :],
                        qrwTf[:, gi * 32:gi * 32 + 32],
                        t2p[:, g % 2, :], start=False, stop=True,
                        tile_position=(0, g * 32))
                tx_blk[blk] = tx_tiles
                return l_ps

            def stageB(blk, l_ps):
                """Masked softmax for block blk (row mask folded into 1/den)."""
                nmx = smallp.tile([128, 1], F32, tag="nmx", name="nmx")
                nc.vector.tensor_reduce(nmx, l_ps, axis=AX.X, op=ALU.max,
                                        negate=True)
                den = smallp.tile([128, 1], F32, tag="den", name="den")
                attn_e = attnp.tile([128, 512], F16, tag="ae", name="attn_e")
                nc.scalar.activation(attn_e, l_ps, ACTF.Exp, bias=nmx,
                                     scale=1.0, accum_out=den)
                rden = smallp.tile([128, 1], F32, tag="rden", name="rden")
                nc.vector.reciprocal(rden, den)
                rdm = smallp.tile([128, 1], F32, tag="rdm", name="rdm")
                nc.vector.tensor_tensor(rdm, rden, rmaskx[:, blk:blk + 1],
                                        op=ALU.mult)
                attn_n = attnp.tile([128, 512], F16, tag="an", name="attn_n")
                nc.vector.tensor_scalar_mul(attn_n, attn_e, rdm)
                attn_blk[blk] = attn_n

            def stageC(blk):
                """Attention transpose + V-path for block blk."""
                attn_n = attn_blk.pop(blk)
                tx_tiles = tx_blk.pop(blk)
                at4_ps = ps_atr.tile([128, 4, 128], F32, tag="atr",
                                     name="at4_ps")
                for g in range(4):
                    nc.tensor.matmul(
                        at4_ps[:, g, :],
                        attn_n[:, g * 128:(g + 1) * 128],
                        eye16,
                        start=True, stop=True)
                az = avz[blk % 2]
                at4v = at4_ps.rearrange("p g (s h) -> p g s h", h=8)
                for gc in range(4):
                    nc.vector.tensor_copy(
                        az[:, gc::4, 8 * gc:8 * gc + 8],
                        at4v[:, gc, gc::4, :])
                gt2_ps = ps_g.tile([128, DX], F32, tag="gt", name="gt2_ps")
                for g in range(4):
                    for kk in range(4):
                        nc.tensor.matmul(
                            gt2_ps[32 * g:32 * (g + 1), :],
                            az[:, 4 * g + kk, :],
                            tx_tiles[g][:, kk, :],
                            start=(kk == 0), stop=(kk == 3),
                            tile_position=(0, 32 * g))
                gt2sb = smallp.tile([128, DX], F16, tag="g2", name="gt2sb")
                nc.scalar.activation(gt2sb, gt2_ps, ACTF.Copy)
                bsl = slice(blk * 16, (blk + 1) * 16)
                for c in range(2):
                    tr = ps_atr.tile([128, 128], F32, tag="atr", name="tr")
                    nc.tensor.matmul(tr, gt2sb[:, c * 128:(c + 1) * 128],
                                     eye16, start=True, stop=True)
                    nc.vector.tensor_copy(
                        gall[:, bsl, c * 8:(c + 1) * 8],
                        tr.rearrange("p (s h) -> p s h", h=8))
                tr2 = ps_atr.tile([32, 128], F32, tag="atr", name="tr2")
                nc.tensor.matmul(tr2, gt2sb[:, 256:288], eye16,
                                 start=True, stop=True)
                nc.vector.tensor_copy(
                    gall[0:32, bsl, 16:24],
                    tr2.rearrange("p (s h) -> p s h", h=8))

            def heartbeat(n):
                for _ in range(n):
                    nc.tensor.matmul(warm_ps, eye16, eye16, start=True,
                                     stop=True, skip_group_check=True)

            for blk in range(nblk):
                if blk > 0:
                    heartbeat(8)
                l_ps = stageA(blk)
                if 1 <= blk <= 4:
                    for h in range(2 * (blk - 1), 2 * blk):
                        issue_wfx(h)
                if blk == 3:
                    nc.scalar.dma_start(out=obias, in_=obias_d[:, :])
                stageB(blk, l_ps)
                if blk >= 1:
                    stageC(blk - 1)
            heartbeat(8)
            stageC(nblk - 1)

            # ---------------- output projection ----------------
            out_ps = ps_misc.tile([sc, DOUT], F32, tag="mA", name="out_ps",
                                  bufs=2)
            for h in range(H):
                for c in range(2):
                    nc.tensor.matmul(
                        out_ps,
                        gall[:, :, c * 8 + h],
                        wfx_main[(h, c)],
                        start=(h == 0 and c == 0), stop=False)
                nc.tensor.matmul(
                    out_ps,
                    gall[0:32, :, 16 + h],
                    wfx_r[h],
                    start=False, stop=(h == H - 1))
            out_sb = cp.tile([sc, DOUT], F32, name="out_sb")
            nc.vector.tensor_tensor(out_sb, out_ps, obias, op=ALU.add)
            nc.sync.dma_start(out=out_d[:, :], in_=out_sb)

    nc.finalize()
    return nc


def host_prep(src, tgt, rpe, tgt_padding_mask, in_proj_weight, in_proj_bias,
              out_proj_weight, out_proj_bias, rpe_weight, rpe_bias):
    """Host-side slicing/weight prep. Returns per-core input maps."""
    f = np.float32
    f16 = np.float16
    scale = f(1.0 / np.sqrt(DH))
    src_f = np.ascontiguousarray(np.asarray(src, f).reshape(BS, D)).astype(f16)
    tgtx = np.concatenate(
        [np.asarray(tgt, f).reshape(BS, T, D),
         np.asarray(rpe, f).reshape(BS, T, DR)], axis=-1).astype(f16)
    mask = np.asarray(tgt_padding_mask, bool).reshape(BS, T)
    no_valid = mask.all(-1)
    maskadd = np.where(mask & ~no_valid[:, None], f16(MASKV), f16(0.0))
    rowmask = (~no_valid).astype(f)

    # bm[blk, j, :] : row j's mask at its own group slot, MASKV elsewhere
    nblk_total = BS // 16
    bm = np.full((nblk_total, 16, 4, T), f16(MASKV), f16)
    ma_b = maskadd.reshape(nblk_total, 16, T)
    for j in range(16):
        bm[:, j, j % 4, :] = ma_b[:, j, :]
    bm = bm.reshape(nblk_total, 16, 512)

    # natural layout, group-packed: [ngr, T, 4*DX]
    ngr_total = BS // 4
    txn = np.ascontiguousarray(
        tgtx.reshape(ngr_total, 4, T, DX).transpose(0, 2, 1, 3)
    ).reshape(ngr_total, T, 4 * DX)
    # transposed layout + bm rows: [ngr, DXM, 512]
    txt = np.empty((ngr_total, DXM, 512), f16)
    txt[:, :DX, :] = tgtx.reshape(ngr_total, 4, T, DX).transpose(
        0, 3, 1, 2).reshape(ngr_total, DX, 512)
    txt[:, DX:, :] = bm[np.arange(ngr_total) // 4]

    sidx = np.arange(SC) % 16
    a16x = (np.arange(16)[:, None, None] == sidx[None, :, None]).astype(f16)
    a16x = np.ascontiguousarray(np.broadcast_to(a16x, (16, SC, H)))

    ipw = np.asarray(in_proj_weight, f)
    ipb = np.asarray(in_proj_bias, f)
    opw = np.asarray(out_proj_weight, f)
    opb = np.asarray(out_proj_bias, f)
    rw = np.asarray(rpe_weight, f)
    rb = np.asarray(rpe_bias, f)

    wsrcT = (ipw[:D].T * scale).astype(f16)                      # [d, e]
    bsrc = (ipb[:D] * scale).astype(f)                           # [D]
    wk = ipw[D:2 * D].astype(f16)                                # [e, d]
    rwk = rw[:D].astype(f16)                                     # [e, r]
    wkk = np.ascontiguousarray(
        np.concatenate([wsrcT, wk, rwk], axis=1))                # [256, 544]
    wvx = np.concatenate([ipw[2 * D:3 * D], rw[D:2 * D]], axis=1)  # [e, 288]
    wfx = np.empty((H, DX, DOUT), f)
    for h in range(H):
        hs = slice(h * 32, (h + 1) * 32)
        wfx[h] = (opw[:, hs] @ wvx[hs, :]).T
    wfx = wfx.astype(f16)
    obias_full = (opb + opw @ (ipb[2 * D:3 * D] + rb[D:2 * D])).astype(f)
    obias_all = np.ascontiguousarray(
        rowmask[:, None] * obias_full[None, :]).astype(f)

    # rmaskx[j*8+h, blk] = rowmask for row (blk*16+j), per core
    nblk = SC // 16
    ngr = SC // 4
    in_maps = []
    for c in range(NCORES):
        sl = slice(c * SC, (c + 1) * SC)
        rm = rowmask[sl].reshape(nblk, 16)
        rmx = np.repeat(rm.T, H, axis=0).astype(f)    # [128, nblk]
        fbl = np.ascontiguousarray(np.concatenate(
            [bsrc.reshape(2, 128).T, rmx], axis=1))   # [128, 10]
        in_maps.append({
            "src": src_f[sl],
            "txn": txn[c * ngr:(c + 1) * ngr],
            "txt": txt[c * ngr:(c + 1) * ngr],
            "a16x": a16x,
            "wkk": wkk,
            "wfx": wfx,
            "fbl": fbl,
            "eye": np.eye(128, dtype=f16),
            "obias": obias_all[sl],
        })
    return in_maps


_NC_CACHE = {}


def get_nc(sc=SC):
    if sc not in _NC_CACHE:
        _NC_CACHE[sc] = build(sc)
    return _NC_CACHE[sc]


def run(in_maps, trace=False):
    nc = get_nc(SC)
    return run_bass_kernel_spmd(nc, in_maps, list(range(NCORES)), trace=trace)


def kernel(**inputs):
    in_maps = host_prep(**inputs)
    res = run(in_maps).results
    out = np.concatenate([res[c]["out"] for c in range(NCORES)], axis=0)
    return np.ascontiguousarray(out.reshape(B, S, D))
